# revision 37
# baseline (speedup 1.0000x reference)
"""Circulant-matmul kernel for Trainium2 (8 NeuronCores, SPMD).

Problem: out[b, i, d] = sum_m alpha[(i - m) mod N] * x[b, m, d]
with x: [2, 8192, 32] fp32, alpha: [8192] fp32.

Strategy
--------
Flatten x to X[m, f] with f = b*32 + d (F = 64 columns). Shard the output
token dim across 8 cores: core c computes rows [1024c, 1024c + 1024).
Rotating alpha on the host (alpha_c[k] = alpha[(k + 1024c) % N]) makes every
core's program identical (SPMD).

Per core, out.T = X.T @ W.T runs as 64 full-array accumulating matmuls in
float32r (fp32 storage; the PE rounds operands to ~13 mantissa bits, giving
~1.5e-4 relative error - orders of magnitude inside the resid-var gate).

The matmul contraction index r maps to SBUF partition p = 127 - r on BOTH
operands (sum order is irrelevant), making the weight skew DMA's partition
step +1 (the BIR verifier rejects negative partition steps):

  wbuf[p, j] = alpha_c[(j + p - 127) % N]
    -- filled by strided DMAs: partition p reads contiguous floats of the
       doubled alpha array starting at element (N - 127 + p + j0).

Pair-stationary trick: step j uses the stationary [128, 128] tile
  [ X_j | X_{(j+4) % 64} ]
with moving slice wbuf[:, s_j : s_j + 512], s_j = (-128 j) mod N, so one
matmul accumulates BOTH halves of the core's output:
  psum[0:64,  q] += X_j.T       @ slice -> out.T[f, q]        (i0 = 0)
  psum[64:128,q] += X_{j+4}.T   @ slice -> out.T[f, 512 + q]  (i0 = 512)
(shifting the block index by 4 shifts the weight slice by exactly 512).

The paired stationaries [128, 8192] are built on-chip by the Vector engine
from the compact X buffer (xsrc, [128, 4352] = 64 blocks + 4 wrap-pad
blocks) with strided spread-copies, saving 2 MB of HBM traffic per core.

Whole core output accumulates in ONE psum bank [128, 512] over 64 matmuls;
the host un-permutes the [128, 512] result tile.
"""

import os
import sys

import numpy as np

for _p in ("/opt/trn_rl_repo",):
    if os.path.isdir(_p) and _p not in sys.path:
        sys.path.insert(0, _p)

import concourse.bass as bass
import concourse.tile as tile
from concourse import bacc, bass_utils, mybir
from concourse.vector_clock import ScopedClock

N = 8192          # token axis
P = 128           # SBUF partitions / matmul contraction tile
F = 64            # packed feature dim (B * D = 2 * 32)
NM = N // P       # 64 m-blocks
JW = 8576         # wbuf columns (max slice start 8064 + 512)
NCORES = 8
NI = N // NCORES  # 1024 output rows per core
MM_DTYPE = mybir.dt.float32r
WARMUP_MM = 8     # PE warmup matmuls while input DMAs stream
XSRC_W = (NM + 4) * F  # 4352

_cache = {}


class FastTileContext(tile.TileContext):
    """TileContext whose exit emits only the final drain (with sem waits on
    all outstanding work) and skips the two all-engine barriers + semaphore
    reset (~9 us of kernel tail). Safe here: the kernel preamble clears all
    semaphores at the start of every execution, so a subsequent run of the
    NEFF still sees zeroed semaphores."""

    def _drain_and_barrier(self, tick_clock, wait_clock):
        drain_inst = self.nc.sync.drain()
        wait_clock.add_sem_waits(
            drain_inst.ins, ScopedClock({None: tick_clock.global_clock})
        )
        popped = self.nc._tile_sem_poison_stack.pop()
        assert popped is self._sem_poison


def _slice_start(j):
    return (-P * j) % N


def _build():
    nc = bacc.Bacc(
        "TRN2", target_bir_lowering=False, debug=False, num_devices=NCORES
    )
    xin = nc.dram_tensor("xin", [P, XSRC_W], MM_DTYPE, kind="ExternalInput")
    a2 = nc.dram_tensor("a2", [2 * N + 512], MM_DTYPE, kind="ExternalInput")
    yout = nc.dram_tensor("yout", [P, 512], mybir.dt.float32, kind="ExternalOutput")

    with FastTileContext(nc) as tc:
        with (
            tc.tile_pool(name="sb", bufs=1) as pool,
            tc.tile_pool(name="ps", bufs=1, space="PSUM") as pp,
        ):
            wbuf = pool.tile([P, JW], MM_DTYPE, tag="wbuf")
            xsrc = pool.tile([P, XSRC_W], MM_DTYPE, tag="xsrc")
            xpair = pool.tile([P, P * NM], MM_DTYPE, tag="xpair")
            obuf = pool.tile([P, 512], mybir.dt.float32, tag="obuf")
            wrm = pool.tile([P, 512], mybir.dt.bfloat16, tag="wrm")

            # PE warmup: HAM un-throttles after ~3.4us of sustained PE
            # activity; run throwaway bf16 matmuls on a memset tile while
            # the input DMAs stream, so real matmuls start at 2.4 GHz.
            if WARMUP_MM:
                ps_wrm = pp.tile([P, 512], mybir.dt.float32, tag="ps_wrm")
                nc.gpsimd.memset(wrm[:, :], 0.0)
            for w in range(WARMUP_MM):
                nc.tensor.matmul(
                    ps_wrm[:, :],
                    lhsT=wrm[:, 0:128],
                    rhs=wrm[:, :],
                    start=(w == 0),
                    stop=(w == WARMUP_MM - 1),
                )

            # Matmul order j = 63, 62, ..., 1, 0: the moving slice start
            # s = (-128j) mod N then ascends 128, 256, ..., 8064, wrapping to
            # 0 for the final step - so wbuf chunks stream strictly in
            # consumption order, and xpair blocks are consumed descending.

            # ALL input DMAs go on the single SP HWDGE ring, interleaved in
            # strict PE-consumption order. SDMA fair-shares bandwidth across
            # rings with queued work at packet granularity, so spreading the
            # transfers over two rings delays every completion; one ring
            # drains FIFO and the next-needed chunk always completes first.
            xblk = [(60, 68), (52, 60), (44, 52), (36, 44), (26, 36), (14, 26), (0, 14)]
            wchunks = [(0, 640), (640, 1136), (1136, 1632)] + [
                (1632 + 992 * k, 1632 + 992 * (k + 1)) for k in range(7)
            ]

            def dma_x(blo, bhi):
                nc.sync.dma_start(
                    out=xsrc[:, F * blo : F * bhi],
                    in_=xin.ap()[:, F * blo : F * bhi],
                )

            def dma_w(lo, hi):
                # skewed fill: wbuf[p, j] = a2[N - 127 + p + j]
                nc.sync.dma_start(
                    out=wbuf[:, lo:hi],
                    in_=bass.AP(a2, N - (P - 1) + lo, [[1, P], [1, hi - lo]]),
                )

            dma_x(*xblk[0])
            dma_w(*wchunks[0])
            dma_w(*wchunks[1])
            dma_x(*xblk[1])
            dma_w(*wchunks[2])
            dma_w(*wchunks[3])
            dma_x(*xblk[2])
            dma_w(*wchunks[4])
            dma_x(*xblk[3])
            dma_w(*wchunks[5])
            dma_x(*xblk[4])
            dma_w(*wchunks[6])
            dma_x(*xblk[5])
            dma_w(*wchunks[7])
            dma_x(*xblk[6])
            dma_w(*wchunks[8])
            dma_w(*wchunks[9])

            # Build paired stationaries with DVE spread-copies:
            #   xpair[:, 128j + u]      = xsrc[:, 64j + u]          (u < 64)
            #   xpair[:, 128j + 64 + u] = xsrc[:, 64(j+4) + u]
            # one group per xsrc chunk (group k's sources live in chunks
            # k and k-1), highest group first, two strided copies per group.
            xs = xsrc[:, :]
            xp = xpair[:, :]
            for blo, bhi in xblk:
                if blo >= NM:
                    continue
                nblk = min(bhi, NM) - blo
                for half, off in ((0, 0), (1, 4 * F)):
                    nc.vector.tensor_copy(
                        bass.AP(
                            xp.tensor,
                            xp.offset + 2 * F * blo + F * half,
                            [[P * NM, P], [2 * F, nblk], [1, F]],
                        ),
                        bass.AP(
                            xs.tensor,
                            xs.offset + F * blo + off,
                            [[XSRC_W, P], [F, nblk], [1, F]],
                        ),
                    )

            ps = pp.tile([P, 512], mybir.dt.float32, tag="ps")
            j_seq = list(range(NM - 1, -1, -1))
            for k, j in enumerate(j_seq):
                s = _slice_start(j)
                nc.tensor.matmul(
                    ps[:, :],
                    lhsT=xpair[:, P * j : P * (j + 1)],
                    rhs=wbuf[:, s : s + 512],
                    start=(k == 0),
                    stop=(k == NM - 1),
                )
            # split the PSUM->SBUF copy and output DMA into halves so the
            # first DMA issues while the second half is still copying
            nc.vector.tensor_copy(obuf[:, 0:256], ps[:, 0:256])
            nc.sync.dma_start(out=yout.ap()[:, 0:256], in_=obuf[:, 0:256])
            nc.vector.tensor_copy(obuf[:, 256:512], ps[:, 256:512])
            nc.scalar.dma_start(out=yout.ap()[:, 256:512], in_=obuf[:, 256:512])
    nc.compile()
    return nc


def _prep_in_maps(x, alpha_delta):
    X = np.ascontiguousarray(x.transpose(1, 0, 2).reshape(N, F))
    # Xb[M, p, f] = X[128M + 127 - p, f]   (reversed r-within-block)
    Xb = X.reshape(NM, P, F)[:, ::-1, :]
    Xb = np.concatenate([Xb, Xb[:4]], axis=0)  # wrap pad: X_0..X_3
    xin = np.ascontiguousarray(Xb.transpose(1, 0, 2).reshape(P, XSRC_W))
    in_maps = []
    for c in range(NCORES):
        ac = np.roll(alpha_delta, -NI * c)
        a2 = np.ascontiguousarray(
            np.concatenate([ac, ac, ac[:512]]).astype(np.float32)
        )
        in_maps.append({"xin": xin, "a2": a2})
    return in_maps


def get_nc():
    if "nc" not in _cache:
        _cache["nc"] = _build()
    return _cache["nc"]


def run(x, alpha_delta, **kwargs):
    """Run on hardware; returns (out [2, N, 32], BassKernelResults)."""
    x = np.asarray(x, dtype=np.float32)
    alpha_delta = np.asarray(alpha_delta, dtype=np.float32)
    res = bass_utils.run_bass_kernel_spmd(
        get_nc(), _prep_in_maps(x, alpha_delta), core_ids=list(range(NCORES)),
        **kwargs,
    )
    out = np.empty((N, F), np.float32)
    for c in range(NCORES):
        y = res.results[c]["yout"]  # [128, 512]
        out[c * NI : c * NI + 512, :] = y[:F, :].T
        out[c * NI + 512 : (c + 1) * NI, :] = y[F:, :].T
    out = np.ascontiguousarray(out.reshape(N, 2, 32).transpose(1, 0, 2))
    return out, res


def kernel(x, alpha_delta):
    out, _ = run(x, alpha_delta)
    return out
